# revision 1
# baseline (speedup 1.0000x reference)
"""Multi-head attention (B=4, S=2048, E=1024, H=16, D=64) on 8 TRN2 cores.

Sharding: core c handles batch b = c//2, query half = c%2 (1024 queries).
Each core computes K/V over its batch's full sequence, attention for all 16
heads over its 1024 queries, and the output projection for its output chunk.
Outputs are disjoint -> host gather is concatenation.

The host rotates each core's sequence so its query block is rows 0:1024
(attention is permutation-invariant over keys) and pre-packs every input into
its exact SBUF layout (bf16), so all device DMAs are contiguous block loads.

Precision / PE modes:
- Q/K/V projections, scores and the output projection run bf16 (operand
  rounding ~0.4%, accumulation in fp32 PSUM).
- The probs/V matmul runs fp8e4m3 with the DoubleRow perf mode over key-chunk
  pairs (2x PE throughput): exp() writes probs directly as fp8, and V is kept
  as a dual-fp8 pair (vp_hi + residual vp_lo, accumulated in the same PSUM
  group) so V-quantization error cancels to ~0.1%.
- Wv is pre-scaled x32 on the host so 32*v fits fp8 normal range; Wo absorbs
  the 1/32. ON stays SBUF-resident (no DRAM round-trip); y is stored bf16
  and widened on the host.

Schedule: all per-head tensors stay SBUF-resident. The prologue emits just
enough projection work (hp0 Q-half0/K-chunk0/V-chunks0-1, with xb streamed
per e-chunk) to start the attention pipeline; query-half 0's attention
stream absorbs the remaining projections (cost-paced, deadline-ordered),
query-half 1's stream absorbs query-half 0's output projection and the
deferred Q-half-1 projections, and the output-projection tail is split
(contraction chunks 0-4 banked into SBUF partials seeded with the bias)
so only a 3-chunk tail remains after the last normalize. PV matmuls are
emitted one key-pair late so the PE score stream never blocks the exp chain.
"""

from contextlib import ExitStack

import numpy as np
import ml_dtypes

import concourse.bass as bass
import concourse.tile as tile
from concourse import bacc, mybir
from concourse.bass_utils import run_bass_kernel_spmd

dt = mybir.dt
AF = mybir.ActivationFunctionType
DR = mybir.MatmulPerfMode.DoubleRow

B, S, E, H, D = 4, 2048, 1024, 16, 64
N_CORES = 8
SQ = 1024          # queries per core
P = 128
EC = E // P        # 8 e-chunks
TC = S // P        # 16 key chunks
KCP = TC // 2      # 8 key-chunk pairs (DoubleRow contraction)
QC = SQ // P       # 8 query chunks
HP = H // 2        # 8 head-pairs
NG = H // 4        # 4 head groups (V projection granularity)

WV_SCALE = 32.0
EXP_SCALE = 0.125

F8 = dt.float8e4
BF = dt.bfloat16
F32 = dt.float32


def _emit(nc, tc, xb_d, wqk_d, wv_d, wo_d, bo_d, y_d):
    with ExitStack() as ctx:
        ctx.enter_context(nc.allow_low_precision(
            reason="bf16 softmax-normalize/bias; error budget validated "
                   "against the fp32 reference"))
        const = ctx.enter_context(tc.tile_pool(name="const", bufs=1))
        big = ctx.enter_context(tc.tile_pool(name="big", bufs=1))
        w1 = ctx.enter_context(tc.tile_pool(name="w1", bufs=2))
        w = ctx.enter_context(tc.tile_pool(name="w", bufs=2))
        ut_pool = ctx.enter_context(tc.tile_pool(name="ut", bufs=4))
        ys = ctx.enter_context(tc.tile_pool(name="ys", bufs=8))
        rp = ctx.enter_context(tc.tile_pool(name="rp", bufs=1))
        ps_s = ctx.enter_context(tc.tile_pool(name="ps_s", bufs=2, space="PSUM"))
        ps_o = ctx.enter_context(tc.tile_pool(name="ps_o", bufs=2, space="PSUM"))
        ps_p = ctx.enter_context(tc.tile_pool(name="ps_p", bufs=2, space="PSUM"))

        # ---- persistent SBUF tensors ----
        xb = big.tile([P, EC, S], BF)              # x^T, e on (ec, p)
        wv = big.tile([P, EC, E], BF)              # 32 * Wv^T
        wo = big.tile([P, EC, E], BF)              # Wo^T / 32
        qt = big.tile([P, HP, SQ], BF)             # q, d on partitions
        kt = big.tile([P, HP, S], BF)              # k
        vhi = big.tile([P, NG, KCP, 2, 4, 68], F8)  # fp8(32v) + ones col 64
        vlo = big.tile([P, NG, KCP, 2, 4, 68], F8)  # residual 32v - vhi
        onT = big.tile([P, HP, SQ], BF)            # 32 * attn out
        bo_rep = big.tile([P, E], BF)

        ones_col = const.tile([P, 1], F32)
        nc.vector.memset(ones_col[:], 1.0)
        # only the pad columns (64:68) need zeroing; cols 0:64 are fully
        # written by the V-projection copies.
        vhi_flat = vhi[:].rearrange("p a b c d e -> p (a b c d) e")
        vlo_flat = vlo[:].rearrange("p a b c d e -> p (a b c d) e")
        nc.vector.memset(vhi_flat[:, :, 64:68], 0.0)
        nc.vector.memset(vlo_flat[:, :, 64:68], 0.0)
        nc.vector.tensor_copy(
            vhi_flat[:, :, 64:65],
            ones_col[:, None, :].to_broadcast([P, NG * KCP * 2 * 4, 1]))

        # ---- op lists: (pe_cost_ns, op) pairs ----
        wsb_t = {}

        def proj_qk_ops(hp):
            ops = []
            st = {}

            def wload():
                wsb_t[hp] = w1.tile([P, 2, EC, P], BF, tag="wqk",
                                    name=f"wqk{hp}")
                nc.sync.dma_start(wsb_t[hp][:], wqk_d[hp])
            ops.append((0, wload))

            # Q projection, query-half 0 only (half 1 runs during qh 1)
            def qalloc():
                st["pq"] = ps_p.tile([P, 512], F32, tag="PROJ",
                                     name=f"pq{hp}_0")
            ops.append((0, qalloc))
            for ec in range(EC):
                def qmm(ec=ec):
                    nc.tensor.matmul(
                        st["pq"][:], wsb_t[hp][:, 0, ec],
                        xb[:, ec, 0:512],
                        start=(ec == 0), stop=(ec == EC - 1))
                ops.append((213, qmm))

            def qcopy():
                nc.vector.tensor_copy(qt[:, hp, 0:512], st["pq"][:])
            ops.append((0, qcopy))

            # K projection: four 512-key chunks
            for nk in range(4):
                def kalloc(nk=nk):
                    st["pk"] = ps_p.tile([P, 512], F32, tag="PROJ",
                                         name=f"pk{hp}_{nk}")
                ops.append((0, kalloc))
                for ec in range(EC):
                    def kmm(ec=ec, nk=nk):
                        nc.tensor.matmul(
                            st["pk"][:], wsb_t[hp][:, 1, ec],
                            xb[:, ec, nk * 512:(nk + 1) * 512],
                            start=(ec == 0), stop=(ec == EC - 1))
                    ops.append((213, kmm))

                def kcopy(nk=nk):
                    nc.vector.tensor_copy(
                        kt[:, hp, nk * 512:(nk + 1) * 512], st["pk"][:])
                ops.append((0, kcopy))
            return ops

        def proj_v_ops(g, tcs):
            """V projection for group g (heads 4g..4g+3), key chunks tcs."""
            ops = []
            st = {}
            for tc_i in tcs:
                def valloc(tc_i=tc_i):
                    st["pv"] = ps_p.tile([P, 512], F32, tag="PROJ",
                                         name=f"pv{g}_{tc_i}")
                ops.append((0, valloc))
                for ec in range(EC):
                    def vmm(ec=ec, tc_i=tc_i):
                        nc.tensor.matmul(
                            st["pv"][:, :256],
                            xb[:, ec, tc_i * P:(tc_i + 1) * P],
                            wv[:, ec, g * 256:(g + 1) * 256],
                            start=(ec == 0), stop=(ec == EC - 1))
                    ops.append((107, vmm))

                def vcopy(tc_i=tc_i):
                    dst = (slice(None), g, tc_i // 2, tc_i % 2,
                           slice(None), slice(0, 64))
                    src = st["pv"][:, :256].rearrange(
                        "p (h d) -> p h d", h=4)
                    nc.vector.tensor_copy(vhi[dst], src)
                    nc.vector.tensor_sub(vlo[dst], src, vhi[dst])
                ops.append((0, vcopy))
            return ops

        def proj_q1_ops(hp):
            """Q projection for query-half 1 (weight chunk re-fetched: the
            streamed wqk tile for hp has been recycled by then)."""
            ops = []
            st = {}

            def wload():
                st["wq1"] = w1.tile([P, EC, P], BF, tag="wq1",
                                    name=f"wq1_{hp}")
                nc.sync.dma_start(st["wq1"][:], wqk_d[hp, :, 0])
            ops.append((0, wload))

            def qalloc():
                st["pq"] = ps_p.tile([P, 512], F32, tag="PROJ",
                                     name=f"pq{hp}_1")
            ops.append((0, qalloc))
            for ec in range(EC):
                def qmm(ec=ec):
                    nc.tensor.matmul(
                        st["pq"][:], st["wq1"][:, ec], xb[:, ec, 512:1024],
                        start=(ec == 0), stop=(ec == EC - 1))
                ops.append((213, qmm))

            def qcopy():
                nc.vector.tensor_copy(qt[:, hp, 512:1024], st["pq"][:])
            ops.append((0, qcopy))
            return ops

        def outproj_ops(qc, nfs=(0, 1)):
            ops = []
            st = {}
            for nf in nfs:
                def oalloc(nf=nf):
                    st["py"] = ps_p.tile([P, 512], F32, tag="PROJ",
                                         name=f"py{qc}_{nf}")
                ops.append((0, oalloc))
                for o in range(EC):
                    def omm(o=o, nf=nf):
                        nc.tensor.matmul(
                            st["py"][:], onT[:, o, qc * P:(qc + 1) * P],
                            wo[:, o, nf * 512:(nf + 1) * 512],
                            start=(o == 0), stop=(o == EC - 1))
                    ops.append((213, omm))

                def ostore(nf=nf):
                    y_sb = w.tile([P, 512], BF, tag="ysb")
                    nc.vector.tensor_add(
                        y_sb[:], st["py"][:],
                        bo_rep[:, nf * 512:(nf + 1) * 512])
                    nc.sync.dma_start(
                        y_d[qc * P:(qc + 1) * P, nf * 512:(nf + 1) * 512],
                        y_sb[:])
                ops.append((0, ostore))
            return ops

        ysum_t = {}
        O_SPLIT = 5

        def outproj_part1_ops(qc, nf):
            """First O_SPLIT contraction chunks of outproj(qc, nf), banked
            into an SBUF partial (seeded with the bias)."""
            ops = []
            st = {}

            def oalloc():
                st["py"] = ps_p.tile([P, 512], F32, tag="PROJ",
                                     name=f"pyA{qc}_{nf}")
            ops.append((0, oalloc))
            for o in range(O_SPLIT):
                def omm(o=o):
                    nc.tensor.matmul(
                        st["py"][:], onT[:, o, qc * P:(qc + 1) * P],
                        wo[:, o, nf * 512:(nf + 1) * 512],
                        start=(o == 0), stop=(o == O_SPLIT - 1))
                ops.append((213, omm))

            def obank():
                ysum_t[(qc, nf)] = ys.tile([P, 512], BF, tag="ysum",
                                           name=f"ys{qc}_{nf}")
                nc.vector.tensor_add(
                    ysum_t[(qc, nf)][:], st["py"][:],
                    bo_rep[:, nf * 512:(nf + 1) * 512])
            ops.append((0, obank))
            return ops

        def outproj_part2_ops(qc, nf):
            ops = []
            st = {}

            def oalloc():
                st["py"] = ps_p.tile([P, 512], F32, tag="PROJ",
                                     name=f"pyB{qc}_{nf}")
            ops.append((0, oalloc))
            for o in range(O_SPLIT, EC):
                def omm(o=o):
                    nc.tensor.matmul(
                        st["py"][:], onT[:, o, qc * P:(qc + 1) * P],
                        wo[:, o, nf * 512:(nf + 1) * 512],
                        start=(o == O_SPLIT), stop=(o == EC - 1))
                ops.append((213, omm))

            def ostore():
                y_sb = w.tile([P, 512], BF, tag="ysb")
                nc.vector.tensor_add(
                    y_sb[:], st["py"][:], ysum_t[(qc, nf)][:])
                nc.sync.dma_start(
                    y_d[qc * P:(qc + 1) * P, nf * 512:(nf + 1) * 512],
                    y_sb[:])
            ops.append((0, ostore))
            return ops

        def emit_attn(qh, hp, aux):
            """Attention for (query half qh, head pair hp); aux ops are
            interleaved at kc granularity to fill PE gaps."""
            g = hp // 2
            ha, hb = 2 * (hp % 2), 2 * (hp % 2) + 1
            qs = slice(qh * 512, (qh + 1) * 512)
            po_a = ps_o.tile([68, 512], F32, tag="po")
            po_b = ps_o.tile([68, 512], F32, tag="po")
            total_cost = sum(c for c, _ in aux) or 1
            n_emit = 0
            cum = 0
            uts = {}

            def emit_pv(kcp):
                ut = uts.pop(kcp)
                for po, h in ((po_a, ha), (po_b, hb)):
                    nc.tensor.matmul(
                        po[:], vhi[:, g, kcp, :, h], ut[:, :, h % 2],
                        start=(kcp == 0), stop=False, perf_mode=DR)
                    nc.tensor.matmul(
                        po[:], vlo[:, g, kcp, :, h], ut[:, :, h % 2],
                        start=False, stop=(kcp == KCP - 1), perf_mode=DR)

            for kc in range(TC):
                kcp, sub = kc // 2, kc % 2
                if sub == 0:
                    uts[kcp] = ut_pool.tile([P, 2, 2, 512], F8, tag="ut", name="ut")
                ut = uts[kcp]
                # PV for pair j is emitted one pair late so it never blocks
                # the scores that feed exp; the sc pool's WAR lag (2 kc)
                # then paces the PE score stream to ACT's rate.
                sc = ps_s.tile([P, 2, 512], F32, tag="S", name="sc")
                nc.tensor.matmul(
                    sc[:, 0], kt[0:64, hp, kc * P:(kc + 1) * P],
                    qt[0:64, hp, qs], start=True, stop=True)
                nc.tensor.matmul(
                    sc[:, 1], kt[64:128, hp, kc * P:(kc + 1) * P],
                    qt[64:128, hp, qs], start=True, stop=True)
                nc.scalar.activation(ut[:, sub], sc[:], AF.Exp,
                                     scale=EXP_SCALE)
                if kc % 2 == 1 and kcp >= 1:
                    emit_pv(kcp - 1)
                target = total_cost * min(kc + 2, TC) // TC
                while n_emit < len(aux) and cum < target:
                    cum += aux[n_emit][0]
                    aux[n_emit][1]()
                    n_emit += 1
            emit_pv(KCP - 1)
            while n_emit < len(aux):
                aux[n_emit][1]()
                n_emit += 1

            # normalize; row 64 of po_x is the softmax denominator.
            # partition_broadcast only writes correctly from base 0:
            # broadcast into a full tile, slice at read time.
            rcp = rp.tile([1, 1024], BF, tag="rcp")
            nc.vector.reciprocal(rcp[:, 0:512], po_a[64:65, :])
            nc.vector.reciprocal(rcp[:, 512:1024], po_b[64:65, :])
            brec = w.tile([P, 1024], BF, tag="brec")
            nc.gpsimd.partition_broadcast(brec[:], rcp[:])
            nc.vector.tensor_mul(
                onT[0:64, hp, qs], po_a[0:64, :], brec[0:64, 0:512])
            nc.vector.tensor_mul(
                onT[64:128, hp, qs], po_b[0:64, :], brec[64:128, 512:1024])

        # ---- schedule ----
        # Prologue: minimal work to start the attention stream -- wqk0 DMA
        # first, xb streamed per e-chunk, then hp0's Q(half 0), K chunk 0 and
        # V key-chunks 0-1. Everything else rides inside attention blocks.
        qk0 = proj_qk_ops(0)
        qk0[0][1]()                           # wqk[0] DMA
        for ec in range(EC):                  # xb split so PE starts early
            nc.sync.dma_start(xb[:, ec], xb_d[:, ec])
        nc.sync.dma_start(wv[:], wv_d)
        bo_one = const.tile([1, E], BF)
        nc.sync.dma_start(bo_one[:], bo_d)
        nc.gpsimd.partition_broadcast(bo_rep[:], bo_one[:])
        for _, op in qk0[1:21]:               # Q half 0 + K chunk 0 complete
            op()
        k_rest = qk0[21:]
        v0 = proj_v_ops(0, range(TC))
        for _, op in v0[:2 * (EC + 2)]:       # tc0, tc1

            op()

        # query-half 0 blocks: hp0 absorbs the rest of its own K/V
        # (need-ordered), later blocks absorb Q/K of hp+1 and V halves of
        # later groups; Q half-1 of hp 0/1 rides in the last two blocks.
        def interleave(a, b):
            """a and b are op-lists; interleave at sub-chunk granularity."""
            out = []
            ia = ib = 0
            ca = sum(c for c, _ in a) or 1
            cb = sum(c for c, _ in b) or 1
            cum_a = cum_b = 0
            while ia < len(a) or ib < len(b):
                if ib >= len(b) or (ia < len(a)
                                    and cum_a * cb <= cum_b * ca):
                    cum_a += a[ia][0]
                    out.append(a[ia])
                    ia += 1
                else:
                    cum_b += b[ib][0]
                    out.append(b[ib])
                    ib += 1
            return out

        for hp in range(HP):
            if hp == 0:
                aux = interleave(k_rest, v0[2 * (EC + 2):])
                aux += proj_qk_ops(1)
            else:
                aux = []
                if hp + 1 < HP:
                    aux += proj_qk_ops(hp + 1)
                if hp == 1:
                    aux = interleave(proj_v_ops(1, range(TC)), aux)
                elif 2 <= hp <= 5:
                    g, half = hp // 2 + 1, hp % 2
                    if g < NG:
                        aux = interleave(
                            proj_v_ops(g, range(half * 8, half * 8 + 8)), aux)
                if hp == 6:
                    aux += proj_q1_ops(0)
                elif hp == 7:
                    aux += proj_q1_ops(1)
            emit_attn(0, hp, aux)

        # query-half 1 blocks: absorb the output projection of query-half 0
        # (one 512-col chunk per block), Q half-1 of hp+2, the wo load, and
        # the first part of query-half 1's own output projection.
        def wo_load():
            nc.sync.dma_start(wo[:], wo_d)
        for hp in range(HP):
            aux = [(0, wo_load)] if hp == 0 else []
            aux += outproj_ops(hp // 2, nfs=(hp % 2,))
            if hp + 2 < HP:
                aux += proj_q1_ops(hp + 2)
            if hp in (5, 6):
                qc = 4 + 2 * (hp - 5)
                for nf in range(2):
                    aux += outproj_part1_ops(qc, nf)
                    aux += outproj_part1_ops(qc + 1, nf)
            emit_attn(1, hp, aux)
        for qc in range(4, QC):
            for nf in range(2):
                for _, op in outproj_part2_ops(qc, nf):
                    op()


def _build_kernel(reps=1):
    nc = bacc.Bacc("TRN2", target_bir_lowering=False, debug=False,
                   num_devices=N_CORES)
    xb_d = nc.dram_tensor("xb", [P, EC, S], BF, kind="ExternalInput").ap()
    wqk_d = nc.dram_tensor("wqk", [HP, P, 2, EC, P], BF,
                           kind="ExternalInput").ap()
    wv_d = nc.dram_tensor("wv", [P, EC, E], BF, kind="ExternalInput").ap()
    wo_d = nc.dram_tensor("wo", [P, EC, E], BF, kind="ExternalInput").ap()
    bo_d = nc.dram_tensor("bo", [1, E], BF, kind="ExternalInput").ap()
    y_d = nc.dram_tensor("y", [SQ, E], BF, kind="ExternalOutput").ap()

    with tile.TileContext(nc) as tc:
        for _ in range(reps):
            _emit(nc, tc, xb_d, wqk_d, wv_d, wo_d, bo_d, y_d)
    nc.compile()
    return nc


_NC_CACHE = None


def make_in_maps(x, Wq, Wk, Wv, Wo, bo):
    x = np.asarray(x, np.float32)
    bf16 = ml_dtypes.bfloat16

    def pack_w(Wt, scale):
        # [E_in, E_out] -> [P, EC, E_out]
        return np.ascontiguousarray(
            (Wt * scale).reshape(EC, P, E).transpose(1, 0, 2)).astype(bf16)

    wqt = pack_w(np.asarray(Wq, np.float32).T, 1.0)   # [P, EC, E]
    wkt = pack_w(np.asarray(Wk, np.float32).T, 1.0)
    wv_ = pack_w(np.asarray(Wv, np.float32).T, WV_SCALE)
    wo_ = pack_w(np.asarray(Wo, np.float32).T, 1.0 / WV_SCALE)

    # wqk: [HP, P, 2, EC, P]
    wqk = np.empty((HP, P, 2, EC, P), bf16)
    for hp in range(HP):
        wqk[hp, :, 0] = wqt[:, :, hp * P:(hp + 1) * P]
        wqk[hp, :, 1] = wkt[:, :, hp * P:(hp + 1) * P]
    wqk = np.ascontiguousarray(wqk)
    bo_ = np.ascontiguousarray(np.asarray(bo, np.float32).reshape(1, E)).astype(bf16)

    in_maps = []
    for c in range(N_CORES):
        b, half = c // 2, c % 2
        x_rot = np.roll(x[b], -half * SQ, axis=0)   # [S, E]
        xb8 = np.ascontiguousarray(
            x_rot.T.reshape(EC, P, S).transpose(1, 0, 2)).astype(bf16)
        in_maps.append({"xb": xb8, "wqk": wqk, "wv": wv_, "wo": wo_,
                        "bo": bo_})
    return in_maps


def get_nc(reps=1):
    global _NC_CACHE
    if _NC_CACHE is None:
        _NC_CACHE = {}
    if reps not in _NC_CACHE:
        _NC_CACHE[reps] = _build_kernel(reps)
    return _NC_CACHE[reps]


def kernel(x, Wq, Wk, Wv, Wo, bo):
    nc = get_nc()
    in_maps = make_in_maps(x, Wq, Wk, Wv, Wo, bo)
    res = run_bass_kernel_spmd(nc, in_maps, core_ids=list(range(N_CORES)))
    out = np.empty((B, S, E), np.float32)
    for c in range(N_CORES):
        b, half = c // 2, c % 2
        out[b, half * SQ:(half + 1) * SQ, :] = \
            res.results[c]["y"].astype(np.float32)
    return out



# revision 4
# speedup vs baseline: 1.1396x; 1.1396x over previous
"""Multi-head attention (B=4, S=2048, E=1024, H=16, D=64) on 8 TRN2 cores.

Sharding: core c handles batch b = c//2, query half = c%2 (1024 queries).
K/V projections are split across the batch pair: each core projects K and V
only for its OWN 1024 sequence positions, then the pair exchanges halves via
pairwise AllGather collectives (replica groups [[0,1],[2,3],[4,5],[6,7]]),
halving the K/V projection matmul work per core. The exchange is SPMD-clean:
both cores of a pair load BOTH gathered halves into global sequence order
(the own half round-trips through the collective and lands bit-identical),
so attention sees keys in global order on every core; softmax is
permutation-invariant over keys so this matches the rotated-query layout.
Outputs are disjoint -> host gather is concatenation.

The host rotates each core's x so its query block is rows 0:1024 and
pre-packs every input into its exact SBUF layout (bf16); xb only carries the
core's OWN 1024 positions (K/V for the other half arrive via the exchange).

Precision / PE modes (identical numerics to the all-local variant):
- Q/K/V projections, scores and the output projection run bf16 (operand
  rounding ~0.4%, accumulation in fp32 PSUM).
- The probs/V matmul runs fp8e4m3 with the DoubleRow perf mode over key-chunk
  pairs (2x PE throughput): exp() writes probs directly as fp8, and V is kept
  as a dual-fp8 pair (vp_hi + residual vp_lo, accumulated in the same PSUM
  group) so V-quantization error cancels to ~0.1%.
- Wv is pre-scaled x32 on the host so 32*v fits fp8 normal range; Wo absorbs
  the 1/32. ON stays SBUF-resident (no DRAM round-trip); y is stored bf16
  and widened on the host.

Schedule: prologue emits hp0's Q/K-own + K-exchange and g0's V-own +
V-exchange so the attention pipeline starts ~12us in; the remaining
projections and their exchanges ride cost-paced inside query-half 0's
attention blocks (deadline-ordered so every collective lands a block ahead
of its consumer), query-half 1's blocks absorb query-half 0's output
projection, and the output-projection tail is split (contraction chunks 0-4
banked into SBUF partials seeded with the bias) so only a 3-chunk tail
remains after the last normalize. PV matmuls are emitted one key-pair late
so the PE score stream never blocks the exp chain.
"""

from contextlib import ExitStack

import numpy as np
import ml_dtypes

import concourse.bass as bass
import concourse.tile as tile
from concourse import bacc, mybir
from concourse.bass_utils import run_bass_kernel_spmd

dt = mybir.dt
AF = mybir.ActivationFunctionType
DR = mybir.MatmulPerfMode.DoubleRow

B, S, E, H, D = 4, 2048, 1024, 16, 64
N_CORES = 8
SQ = 1024          # queries per core (= own sequence positions)
P = 128
EC = E // P        # 8 e-chunks
TC = S // P        # 16 key chunks (full sequence)
TCH = SQ // P      # 8 own-half key chunks
KCP = TC // 2      # 8 key-chunk pairs (DoubleRow contraction)
QC = SQ // P       # 8 query chunks
HP = H // 2        # 8 head-pairs
NG = H // 4        # 4 head groups (V projection granularity)

GROUPS = [[0, 1], [2, 3], [4, 5], [6, 7]]

WV_SCALE = 32.0
EXP_SCALE = 0.125

F8 = dt.float8e4
BF = dt.bfloat16
F32 = dt.float32

_EMIT_COUNTER = [0]


def _emit(nc, tc, xb_d, wqk_d, wv_d, wo_d, bo_d, y_d):
    rep = _EMIT_COUNTER[0]
    _EMIT_COUNTER[0] += 1
    with ExitStack() as ctx:
        ctx.enter_context(nc.allow_low_precision(
            reason="bf16 softmax-normalize/bias; error budget validated "
                   "against the fp32 reference"))
        const = ctx.enter_context(tc.tile_pool(name="const", bufs=1))
        big = ctx.enter_context(tc.tile_pool(name="big", bufs=1))
        w1 = ctx.enter_context(tc.tile_pool(name="w1", bufs=2))
        w = ctx.enter_context(tc.tile_pool(name="w", bufs=2))
        ut_pool = ctx.enter_context(tc.tile_pool(name="ut", bufs=4))
        ys = ctx.enter_context(tc.tile_pool(name="ys", bufs=8))
        rp = ctx.enter_context(tc.tile_pool(name="rp", bufs=1))
        ps_s = ctx.enter_context(tc.tile_pool(name="ps_s", bufs=2, space="PSUM"))
        ps_o = ctx.enter_context(tc.tile_pool(name="ps_o", bufs=2, space="PSUM"))
        ps_p = ctx.enter_context(tc.tile_pool(name="ps_p", bufs=2, space="PSUM"))

        # ---- persistent SBUF tensors ----
        xb = big.tile([P, EC, SQ], BF)             # x^T (own half), e on (ec, p)
        wv = big.tile([P, EC, E], BF)              # 32 * Wv^T
        wo = big.tile([P, EC, E], BF)              # Wo^T / 32
        qt = big.tile([P, HP, SQ], BF)             # q, d on partitions
        kt = big.tile([P, HP, S], BF)              # k (global seq order)
        vhi = big.tile([P, NG, KCP, 2, 4, 68], F8)  # fp8(32v) + ones col 64
        vlo = big.tile([P, NG, KCP, 2, 4, 68], F8)  # residual 32v - vhi
        onT = big.tile([P, HP, SQ], BF)            # 32 * attn out
        bo_rep = big.tile([P, E], BF)

        ones_col = const.tile([P, 1], F32)
        nc.vector.memset(ones_col[:], 1.0)
        # only the pad columns (64:68) need zeroing; cols 0:64 are fully
        # written by the V-projection copies / exchange loads. The exchanged
        # half's pad+ones blocks are overwritten by the gather with the
        # partner's identical constants.
        vhi_flat = vhi[:].rearrange("p a b c d e -> p (a b c d) e")
        vlo_flat = vlo[:].rearrange("p a b c d e -> p (a b c d) e")
        nc.vector.memset(vhi_flat[:, :, 64:68], 0.0)
        nc.vector.memset(vlo_flat[:, :, 64:68], 0.0)
        nc.vector.tensor_copy(
            vhi_flat[:, :, 64:65],
            ones_col[:, None, :].to_broadcast([P, NG * KCP * 2 * 4, 1]))

        # ---- op lists: (pe_cost_ns, op) pairs ----
        wsb_t = {}

        def k_exchange_ops(hp):
            """Pairwise AllGather of kt[:, hp, 0:SQ] (own half) -> kt full."""
            kin = nc.dram_tensor(f"kin{rep}_{hp}", [P, SQ], BF).ap()
            kout = nc.dram_tensor(f"kout{rep}_{hp}", [2, P, SQ], BF).ap()
            ops = []

            def send():
                nc.sync.dma_start(kin, kt[:, hp, 0:SQ])
            ops.append((0, send))

            def cc():
                nc.gpsimd.collective_compute(
                    "AllGather", mybir.AluOpType.bypass,
                    replica_groups=GROUPS,
                    ins=[kin.opt()], outs=[kout.opt()])
            ops.append((0, cc))

            def load():
                nc.sync.dma_start(kt[:, hp, 0:SQ], kout[0])
                nc.sync.dma_start(kt[:, hp, SQ:S], kout[1])
            ops.append((0, load))
            return ops

        def v_exchange_ops(g):
            """Pairwise AllGather of vhi/vlo[:, g, 0:KCP//2] -> full kcp."""
            HB = KCP // 2
            vin = nc.dram_tensor(f"vin{rep}_{g}", [P, 2, HB, 2, 4, 68], F8).ap()
            vout = nc.dram_tensor(f"vout{rep}_{g}", [2, P, 2, HB, 2, 4, 68],
                                  F8).ap()
            ops = []

            def send():
                nc.sync.dma_start(vin[:, 0], vhi[:, g, 0:HB])
                nc.sync.dma_start(vin[:, 1], vlo[:, g, 0:HB])
            ops.append((0, send))

            def cc():
                nc.gpsimd.collective_compute(
                    "AllGather", mybir.AluOpType.bypass,
                    replica_groups=GROUPS,
                    ins=[vin.opt()], outs=[vout.opt()])
            ops.append((0, cc))

            def load():
                nc.sync.dma_start(vhi[:, g, 0:HB], vout[0, :, 0])
                nc.sync.dma_start(vhi[:, g, HB:KCP], vout[1, :, 0])
                nc.sync.dma_start(vlo[:, g, 0:HB], vout[0, :, 1])
                nc.sync.dma_start(vlo[:, g, HB:KCP], vout[1, :, 1])
            ops.append((0, load))
            return ops

        def proj_qk_ops(hp):
            """Q (query-half 0) + K own-half projection + K exchange."""
            ops = []
            st = {}

            def wload():
                wsb_t[hp] = w1.tile([P, 2, EC, P], BF, tag="wqk",
                                    name=f"wqk{hp}")
                nc.sync.dma_start(wsb_t[hp][:], wqk_d[hp])
            ops.append((0, wload))

            # Q projection, query-half 0 only (half 1 runs during qh 1)
            def qalloc():
                st["pq"] = ps_p.tile([P, 512], F32, tag="PROJ",
                                     name=f"pq{hp}_0")
            ops.append((0, qalloc))
            for ec in range(EC):
                def qmm(ec=ec):
                    nc.tensor.matmul(
                        st["pq"][:], wsb_t[hp][:, 0, ec],
                        xb[:, ec, 0:512],
                        start=(ec == 0), stop=(ec == EC - 1))
                ops.append((213, qmm))

            def qcopy():
                nc.vector.tensor_copy(qt[:, hp, 0:512], st["pq"][:])
            ops.append((0, qcopy))

            # K projection: own half only (two 512-key chunks)
            for nk in range(2):
                def kalloc(nk=nk):
                    st["pk"] = ps_p.tile([P, 512], F32, tag="PROJ",
                                         name=f"pk{hp}_{nk}")
                ops.append((0, kalloc))
                for ec in range(EC):
                    def kmm(ec=ec, nk=nk):
                        nc.tensor.matmul(
                            st["pk"][:], wsb_t[hp][:, 1, ec],
                            xb[:, ec, nk * 512:(nk + 1) * 512],
                            start=(ec == 0), stop=(ec == EC - 1))
                    ops.append((213, kmm))

                def kcopy(nk=nk):
                    nc.vector.tensor_copy(
                        kt[:, hp, nk * 512:(nk + 1) * 512], st["pk"][:])
                ops.append((0, kcopy))
            ops += k_exchange_ops(hp)
            return ops

        def proj_v_ops(g):
            """V projection for group g (heads 4g..4g+3), own half + exchange."""
            ops = []
            st = {}
            for tc_i in range(TCH):
                def valloc(tc_i=tc_i):
                    st["pv"] = ps_p.tile([P, 512], F32, tag="PROJ",
                                         name=f"pv{g}_{tc_i}")
                ops.append((0, valloc))
                for ec in range(EC):
                    def vmm(ec=ec, tc_i=tc_i):
                        nc.tensor.matmul(
                            st["pv"][:, :256],
                            xb[:, ec, tc_i * P:(tc_i + 1) * P],
                            wv[:, ec, g * 256:(g + 1) * 256],
                            start=(ec == 0), stop=(ec == EC - 1))
                    ops.append((107, vmm))

                def vcopy(tc_i=tc_i):
                    dst = (slice(None), g, tc_i // 2, tc_i % 2,
                           slice(None), slice(0, 64))
                    src = st["pv"][:, :256].rearrange(
                        "p (h d) -> p h d", h=4)
                    nc.vector.tensor_copy(vhi[dst], src)
                    nc.vector.tensor_sub(vlo[dst], src, vhi[dst])
                ops.append((0, vcopy))
            ops += v_exchange_ops(g)
            return ops

        def proj_q1_ops(hp):
            """Q projection for query-half 1 (weight chunk re-fetched: the
            streamed wqk tile for hp has been recycled by then)."""
            ops = []
            st = {}

            def wload():
                st["wq1"] = w1.tile([P, EC, P], BF, tag="wq1",
                                    name=f"wq1_{hp}")
                nc.sync.dma_start(st["wq1"][:], wqk_d[hp, :, 0])
            ops.append((0, wload))

            def qalloc():
                st["pq"] = ps_p.tile([P, 512], F32, tag="PROJ",
                                     name=f"pq{hp}_1")
            ops.append((0, qalloc))
            for ec in range(EC):
                def qmm(ec=ec):
                    nc.tensor.matmul(
                        st["pq"][:], st["wq1"][:, ec], xb[:, ec, 512:1024],
                        start=(ec == 0), stop=(ec == EC - 1))
                ops.append((213, qmm))

            def qcopy():
                nc.vector.tensor_copy(qt[:, hp, 512:1024], st["pq"][:])
            ops.append((0, qcopy))
            return ops

        def outproj_ops(qc, nfs=(0, 1)):
            ops = []
            st = {}
            for nf in nfs:
                def oalloc(nf=nf):
                    st["py"] = ps_p.tile([P, 512], F32, tag="PROJ",
                                         name=f"py{qc}_{nf}")
                ops.append((0, oalloc))
                for o in range(EC):
                    def omm(o=o, nf=nf):
                        nc.tensor.matmul(
                            st["py"][:], onT[:, o, qc * P:(qc + 1) * P],
                            wo[:, o, nf * 512:(nf + 1) * 512],
                            start=(o == 0), stop=(o == EC - 1))
                    ops.append((213, omm))

                def ostore(nf=nf):
                    y_sb = w.tile([P, 512], BF, tag="ysb")
                    nc.vector.tensor_add(
                        y_sb[:], st["py"][:],
                        bo_rep[:, nf * 512:(nf + 1) * 512])
                    nc.sync.dma_start(
                        y_d[qc * P:(qc + 1) * P, nf * 512:(nf + 1) * 512],
                        y_sb[:])
                ops.append((0, ostore))
            return ops

        ysum_t = {}
        O_SPLIT = 5

        def outproj_part1_ops(qc, nf):
            """First O_SPLIT contraction chunks of outproj(qc, nf), banked
            into an SBUF partial (seeded with the bias)."""
            ops = []
            st = {}

            def oalloc():
                st["py"] = ps_p.tile([P, 512], F32, tag="PROJ",
                                     name=f"pyA{qc}_{nf}")
            ops.append((0, oalloc))
            for o in range(O_SPLIT):
                def omm(o=o):
                    nc.tensor.matmul(
                        st["py"][:], onT[:, o, qc * P:(qc + 1) * P],
                        wo[:, o, nf * 512:(nf + 1) * 512],
                        start=(o == 0), stop=(o == O_SPLIT - 1))
                ops.append((213, omm))

            def obank():
                ysum_t[(qc, nf)] = ys.tile([P, 512], BF, tag="ysum",
                                           name=f"ys{qc}_{nf}")
                nc.vector.tensor_add(
                    ysum_t[(qc, nf)][:], st["py"][:],
                    bo_rep[:, nf * 512:(nf + 1) * 512])
            ops.append((0, obank))
            return ops

        def outproj_part2_ops(qc, nf):
            ops = []
            st = {}

            def oalloc():
                st["py"] = ps_p.tile([P, 512], F32, tag="PROJ",
                                     name=f"pyB{qc}_{nf}")
            ops.append((0, oalloc))
            for o in range(O_SPLIT, EC):
                def omm(o=o):
                    nc.tensor.matmul(
                        st["py"][:], onT[:, o, qc * P:(qc + 1) * P],
                        wo[:, o, nf * 512:(nf + 1) * 512],
                        start=(o == O_SPLIT), stop=(o == EC - 1))
                ops.append((213, omm))

            def ostore():
                y_sb = w.tile([P, 512], BF, tag="ysb")
                nc.vector.tensor_add(
                    y_sb[:], st["py"][:], ysum_t[(qc, nf)][:])
                nc.sync.dma_start(
                    y_d[qc * P:(qc + 1) * P, nf * 512:(nf + 1) * 512],
                    y_sb[:])
            ops.append((0, ostore))
            return ops

        def emit_attn(qh, hp, aux):
            """Attention for (query half qh, head pair hp); aux ops are
            interleaved at kc granularity to fill PE gaps."""
            g = hp // 2
            ha, hb = 2 * (hp % 2), 2 * (hp % 2) + 1
            qs = slice(qh * 512, (qh + 1) * 512)
            po_a = ps_o.tile([68, 512], F32, tag="po")
            po_b = ps_o.tile([68, 512], F32, tag="po")
            total_cost = sum(c for c, _ in aux) or 1
            n_emit = 0
            cum = 0
            uts = {}

            def emit_pv(kcp):
                ut = uts.pop(kcp)
                for po, h in ((po_a, ha), (po_b, hb)):
                    nc.tensor.matmul(
                        po[:], vhi[:, g, kcp, :, h], ut[:, :, h % 2],
                        start=(kcp == 0), stop=False, perf_mode=DR)
                    nc.tensor.matmul(
                        po[:], vlo[:, g, kcp, :, h], ut[:, :, h % 2],
                        start=False, stop=(kcp == KCP - 1), perf_mode=DR)

            for kc in range(TC):
                kcp, sub = kc // 2, kc % 2
                if sub == 0:
                    uts[kcp] = ut_pool.tile([P, 2, 2, 512], F8, tag="ut", name="ut")
                ut = uts[kcp]
                # PV for pair j is emitted one pair late so it never blocks
                # the scores that feed exp; the sc pool's WAR lag (2 kc)
                # then paces the PE score stream to ACT's rate.
                sc = ps_s.tile([P, 2, 512], F32, tag="S", name="sc")
                nc.tensor.matmul(
                    sc[:, 0], kt[0:64, hp, kc * P:(kc + 1) * P],
                    qt[0:64, hp, qs], start=True, stop=True)
                nc.tensor.matmul(
                    sc[:, 1], kt[64:128, hp, kc * P:(kc + 1) * P],
                    qt[64:128, hp, qs], start=True, stop=True)
                nc.scalar.activation(ut[:, sub], sc[:], AF.Exp,
                                     scale=EXP_SCALE)
                if kc % 2 == 1 and kcp >= 1:
                    emit_pv(kcp - 1)
                target = total_cost * min(kc + 2, TC) // TC
                while n_emit < len(aux) and cum < target:
                    cum += aux[n_emit][0]
                    aux[n_emit][1]()
                    n_emit += 1
            emit_pv(KCP - 1)
            while n_emit < len(aux):
                aux[n_emit][1]()
                n_emit += 1

            # normalize; row 64 of po_x is the softmax denominator.
            # partition_broadcast only writes correctly from base 0:
            # broadcast into a full tile, slice at read time.
            rcp = rp.tile([1, 1024], BF, tag="rcp")
            nc.vector.reciprocal(rcp[:, 0:512], po_a[64:65, :])
            nc.vector.reciprocal(rcp[:, 512:1024], po_b[64:65, :])
            brec = w.tile([P, 1024], BF, tag="brec")
            nc.gpsimd.partition_broadcast(brec[:], rcp[:])
            nc.vector.tensor_mul(
                onT[0:64, hp, qs], po_a[0:64, :], brec[0:64, 0:512])
            nc.vector.tensor_mul(
                onT[64:128, hp, qs], po_b[0:64, :], brec[64:128, 512:1024])

        # ---- schedule ----
        # Prologue: wqk0 DMA first, xb streamed per e-chunk, then hp0's
        # Q(half 0) / K-own / K-exchange and g0's V-own / V-exchange, so the
        # first attention block's collectives land while its scores stream.
        qk0 = proj_qk_ops(0)
        qk0[0][1]()                           # wqk[0] DMA
        for ec in range(EC):                  # xb split so PE starts early
            nc.sync.dma_start(xb[:, ec], xb_d[:, ec])
        nc.sync.dma_start(wv[:], wv_d)
        bo_one = const.tile([1, E], BF)
        nc.sync.dma_start(bo_one[:], bo_d)
        nc.gpsimd.partition_broadcast(bo_rep[:], bo_one[:])
        for _, op in qk0[1:]:                 # Q half 0 + K own + exchange
            op()
        for _, op in proj_v_ops(0):           # V own g0 + exchange
            op()

        # query-half 0 blocks: the remaining Q/K-own projections (+their
        # exchanges), V groups 1-3 (+exchanges) and the deferred Q-half-1
        # projections ride inside the blocks, deadline-ordered: everything a
        # block consumes is emitted at least one block earlier.
        for hp in range(HP):
            if hp == 0:
                aux = proj_qk_ops(1) + proj_v_ops(1)
            elif hp == 1:
                aux = proj_qk_ops(2) + proj_qk_ops(3)
            elif hp == 2:
                aux = proj_qk_ops(4) + proj_v_ops(2)
            elif hp == 3:
                aux = proj_qk_ops(5) + proj_qk_ops(6)
            elif hp == 4:
                aux = proj_qk_ops(7) + proj_v_ops(3)
            elif hp == 5:
                aux = proj_q1_ops(0) + proj_q1_ops(1)
            elif hp == 6:
                aux = proj_q1_ops(2) + proj_q1_ops(3)
            else:
                aux = proj_q1_ops(4) + proj_q1_ops(5)
            emit_attn(0, hp, aux)

        # query-half 1 blocks: absorb the output projection of query-half 0
        # (one 512-col chunk per block), Q half-1 of hp 6/7, the wo load, and
        # the first part of query-half 1's own output projection.
        def wo_load():
            nc.sync.dma_start(wo[:], wo_d)
        for hp in range(HP):
            aux = [(0, wo_load)] if hp == 0 else []
            aux += outproj_ops(hp // 2, nfs=(hp % 2,))
            if hp == 0:
                aux += proj_q1_ops(6)
            elif hp == 1:
                aux += proj_q1_ops(7)
            if hp in (5, 6):
                qc = 4 + 2 * (hp - 5)
                for nf in range(2):
                    aux += outproj_part1_ops(qc, nf)
                    aux += outproj_part1_ops(qc + 1, nf)
            emit_attn(1, hp, aux)
        for qc in range(4, QC):
            for nf in range(2):
                for _, op in outproj_part2_ops(qc, nf):
                    op()


def _build_kernel(reps=1):
    nc = bacc.Bacc("TRN2", target_bir_lowering=False, debug=False,
                   num_devices=N_CORES)
    xb_d = nc.dram_tensor("xb", [P, EC, SQ], BF, kind="ExternalInput").ap()
    wqk_d = nc.dram_tensor("wqk", [HP, P, 2, EC, P], BF,
                           kind="ExternalInput").ap()
    wv_d = nc.dram_tensor("wv", [P, EC, E], BF, kind="ExternalInput").ap()
    wo_d = nc.dram_tensor("wo", [P, EC, E], BF, kind="ExternalInput").ap()
    bo_d = nc.dram_tensor("bo", [1, E], BF, kind="ExternalInput").ap()
    y_d = nc.dram_tensor("y", [SQ, E], BF, kind="ExternalOutput").ap()

    with tile.TileContext(nc) as tc:
        for _ in range(reps):
            _emit(nc, tc, xb_d, wqk_d, wv_d, wo_d, bo_d, y_d)
    nc.compile()
    return nc


_NC_CACHE = None


def make_in_maps(x, Wq, Wk, Wv, Wo, bo):
    x = np.asarray(x, np.float32)
    bf16 = ml_dtypes.bfloat16

    def pack_w(Wt, scale):
        # [E_in, E_out] -> [P, EC, E_out]
        return np.ascontiguousarray(
            (Wt * scale).reshape(EC, P, E).transpose(1, 0, 2)).astype(bf16)

    wqt = pack_w(np.asarray(Wq, np.float32).T, 1.0)   # [P, EC, E]
    wkt = pack_w(np.asarray(Wk, np.float32).T, 1.0)
    wv_ = pack_w(np.asarray(Wv, np.float32).T, WV_SCALE)
    wo_ = pack_w(np.asarray(Wo, np.float32).T, 1.0 / WV_SCALE)

    # wqk: [HP, P, 2, EC, P]
    wqk = np.empty((HP, P, 2, EC, P), bf16)
    for hp in range(HP):
        wqk[hp, :, 0] = wqt[:, :, hp * P:(hp + 1) * P]
        wqk[hp, :, 1] = wkt[:, :, hp * P:(hp + 1) * P]
    wqk = np.ascontiguousarray(wqk)
    bo_ = np.ascontiguousarray(np.asarray(bo, np.float32).reshape(1, E)).astype(bf16)

    in_maps = []
    for c in range(N_CORES):
        b, half = c // 2, c % 2
        x_own = x[b, half * SQ:(half + 1) * SQ]     # [SQ, E]
        xb8 = np.ascontiguousarray(
            x_own.T.reshape(EC, P, SQ).transpose(1, 0, 2)).astype(bf16)
        in_maps.append({"xb": xb8, "wqk": wqk, "wv": wv_, "wo": wo_,
                        "bo": bo_})
    return in_maps


def get_nc(reps=1):
    global _NC_CACHE
    if _NC_CACHE is None:
        _NC_CACHE = {}
    if reps not in _NC_CACHE:
        _NC_CACHE[reps] = _build_kernel(reps)
    return _NC_CACHE[reps]


def kernel(x, Wq, Wk, Wv, Wo, bo):
    nc = get_nc()
    in_maps = make_in_maps(x, Wq, Wk, Wv, Wo, bo)
    res = run_bass_kernel_spmd(nc, in_maps, core_ids=list(range(N_CORES)))
    out = np.empty((B, S, E), np.float32)
    for c in range(N_CORES):
        b, half = c // 2, c % 2
        out[b, half * SQ:(half + 1) * SQ, :] = \
            res.results[c]["y"].astype(np.float32)
    return out


# revision 13
# speedup vs baseline: 1.1400x; 1.0004x over previous
"""Multi-head attention (B=4, S=2048, E=1024, H=16, D=64) on 8 TRN2 cores.

Sharding: core c handles batch b = c//2, query half = c%2 (1024 queries).
K/V projections are split across the batch pair: each core projects K and V
only for its OWN 1024 sequence positions, then the pair exchanges halves via
pairwise AllGather collectives (replica groups [[0,1],[2,3],[4,5],[6,7]]),
halving the K/V projection matmul work per core. The exchange is SPMD-clean:
both cores of a pair load BOTH gathered halves into global sequence order
(the own half round-trips through the collective and lands bit-identical),
so attention sees keys in global order on every core; softmax is
permutation-invariant over keys so this matches the rotated-query layout.
Outputs are disjoint -> host gather is concatenation.

The host rotates each core's x so its query block is rows 0:1024 and
pre-packs every input into its exact SBUF layout (bf16); xb only carries the
core's OWN 1024 positions (K/V for the other half arrive via the exchange).

Precision / PE modes:
- Q/K/V projections and the output projection run bf16 (operand rounding
  ~0.4%, accumulation in fp32 PSUM).
- Scores run fp8e4m3 DoubleRow: q and k are split PSUM->dual-fp8 (hi +
  residual lo, ~bf16-equivalent precision; 4x pre-scale folded into Wq/Wk so
  the duals sit in fp8 normal range, exp scale absorbs the 1/16). Per head,
  k8 packs hi/lo across partition halves and q8 duplicates q(s,d) across
  both halves, so ONE DR matmul contracts (khi+klo)x(qhi+qlo) per key chunk
  -- half the bf16 score cycles.
- The probs/V matmul runs fp8e4m3 with the DoubleRow perf mode over key-chunk
  pairs (2x PE throughput): exp() writes probs directly as fp8, and V is kept
  as a dual-fp8 pair (vp_hi + residual vp_lo, accumulated in the same PSUM
  group) so V-quantization error cancels to ~0.1%.
- Wv is pre-scaled x32 on the host so 32*v fits fp8 normal range; Wo absorbs
  the 1/32. ON stays SBUF-resident (no DRAM round-trip); y is stored bf16
  and widened on the host.

Schedule: prologue emits hp0's Q/K-own + K-exchange and g0's V-own +
V-exchange so the attention pipeline starts ~12us in; the remaining
projections and their exchanges ride cost-paced inside query-half 0's
attention blocks (deadline-ordered so every collective lands a block ahead
of its consumer), query-half 1's blocks absorb query-half 0's output
projection, and the output-projection tail is split (contraction chunks 0-4
banked into SBUF partials seeded with the bias) so only a 3-chunk tail
remains after the last normalize. PV matmuls are emitted one key-pair late
so the PE score stream never blocks the exp chain.
"""

from contextlib import ExitStack

import numpy as np
import ml_dtypes

import concourse.bass as bass
import concourse.tile as tile
from concourse import bacc, mybir
from concourse.bass_utils import run_bass_kernel_spmd

dt = mybir.dt
AF = mybir.ActivationFunctionType
DR = mybir.MatmulPerfMode.DoubleRow

B, S, E, H, D = 4, 2048, 1024, 16, 64
N_CORES = 8
SQ = 1024          # queries per core (= own sequence positions)
P = 128
EC = E // P        # 8 e-chunks
TC = S // P        # 16 key chunks (full sequence)
TCH = SQ // P      # 8 own-half key chunks
KCP = TC // 2      # 8 key-chunk pairs (DoubleRow contraction)
QC = SQ // P       # 8 query chunks
HP = H // 2        # 8 head-pairs
NG = H // 4        # 4 head groups (V projection granularity)

GROUPS = [[0, 1], [2, 3], [4, 5], [6, 7]]

WV_SCALE = 32.0
QK_SCALE = 4.0     # folded into Wq/Wk on the host; exp scale absorbs 1/16
EXP_SCALE = 0.125 / (QK_SCALE * QK_SCALE)

F8 = dt.float8e4
BF = dt.bfloat16
F32 = dt.float32

_EMIT_COUNTER = [0]


def _emit(nc, tc, xb_d, wqk_d, wv_d, wo_d, bo_d, y_d):
    rep = _EMIT_COUNTER[0]
    _EMIT_COUNTER[0] += 1
    with ExitStack() as ctx:
        ctx.enter_context(nc.allow_low_precision(
            reason="bf16 softmax-normalize/bias; error budget validated "
                   "against the fp32 reference"))
        const = ctx.enter_context(tc.tile_pool(name="const", bufs=1))
        big = ctx.enter_context(tc.tile_pool(name="big", bufs=1))
        w1 = ctx.enter_context(tc.tile_pool(name="w1", bufs=2))
        w = ctx.enter_context(tc.tile_pool(name="w", bufs=2))
        ut_pool = ctx.enter_context(tc.tile_pool(name="ut", bufs=4))
        ys = ctx.enter_context(tc.tile_pool(name="ys", bufs=8))
        rp = ctx.enter_context(tc.tile_pool(name="rp", bufs=1))
        ps_s = ctx.enter_context(tc.tile_pool(name="ps_s", bufs=2, space="PSUM"))
        ps_o = ctx.enter_context(tc.tile_pool(name="ps_o", bufs=2, space="PSUM"))
        ps_p = ctx.enter_context(tc.tile_pool(name="ps_p", bufs=2, space="PSUM"))

        # ---- persistent SBUF tensors ----
        xb = big.tile([P, EC, SQ], BF)             # x^T (own half), e on (ec, p)
        wv = big.tile([P, EC, E], BF)              # 32 * Wv^T
        wo = big.tile([P, EC, E], BF)              # Wo^T / 32
        # fp8 dual (hi + residual lo) Q/K for DoubleRow score matmuls.
        # k8 per head: one partition half holds hi(d), the other lo(d) --
        # the half assignment flips with head parity so every split sub
        # reads both inputs at the same partition base (walrus constraint).
        # q8 duplicates each head's q(s, d) across both partition halves so
        # one DR matmul contracts (khi+klo) x (qhi+qlo) in 256 cycles.
        k8 = big.tile([P, H, S], F8)               # k (global seq order)
        q8 = big.tile([P, 2, H, SQ], F8)           # s dim = hi/lo
        vhi = big.tile([P, NG, KCP, 2, 4, 68], F8)  # fp8(32v) + ones col 64
        vlo = big.tile([P, NG, KCP, 2, 4, 68], F8)  # residual 32v - vhi
        onT = big.tile([P, HP, SQ], BF)            # 32 * attn out
        bo_rep = big.tile([P, E], BF)

        ones_col = const.tile([P, 1], F32)
        nc.vector.memset(ones_col[:], 1.0)
        # only the pad columns (64:68) need zeroing; cols 0:64 are fully
        # written by the V-projection copies / exchange loads. The exchanged
        # half's pad+ones blocks are overwritten by the gather with the
        # partner's identical constants.
        vhi_flat = vhi[:].rearrange("p a b c d e -> p (a b c d) e")
        vlo_flat = vlo[:].rearrange("p a b c d e -> p (a b c d) e")
        nc.vector.memset(vhi_flat[:, :, 64:68], 0.0)
        nc.vector.memset(vlo_flat[:, :, 64:68], 0.0)
        nc.vector.tensor_copy(
            vhi_flat[:, :, 64:65],
            ones_col[:, None, :].to_broadcast([P, NG * KCP * 2 * 4, 1]))

        # ---- op lists: (pe_cost_ns, op) pairs ----
        wsb_t = {}

        def _q_split(hp, pq, cq):
            """PSUM q (2 heads on partition halves) -> q8 hi/lo duals,
            duplicated across both partition halves."""
            ha, hb = 2 * hp, 2 * hp + 1
            nc.vector.tensor_copy(q8[0:64, 0, ha, cq], pq[0:64])
            nc.vector.tensor_copy(q8[64:128, 0, ha, cq], pq[0:64])
            nc.vector.tensor_sub(
                q8[0:64, 1, ha, cq], pq[0:64], q8[0:64, 0, ha, cq])
            nc.vector.tensor_copy(q8[64:128, 1, ha, cq], q8[0:64, 1, ha, cq])
            nc.vector.tensor_copy(q8[64:128, 0, hb, cq], pq[64:128])
            nc.vector.tensor_copy(q8[0:64, 0, hb, cq], pq[64:128])
            nc.vector.tensor_sub(
                q8[64:128, 1, hb, cq], pq[64:128], q8[64:128, 0, hb, cq])
            nc.vector.tensor_copy(q8[0:64, 1, hb, cq], q8[64:128, 1, hb, cq])

        def k_exchange_ops(hp):
            """Pairwise AllGather of k8[:, 2hp:2hp+2, 0:SQ] (own half)."""
            ha, hb = 2 * hp, 2 * hp + 1
            kin = nc.dram_tensor(f"kin{rep}_{hp}", [P, 2, SQ], F8).ap()
            kout = nc.dram_tensor(f"kout{rep}_{hp}", [2, P, 2, SQ], F8).ap()
            ops = []

            def send():
                nc.sync.dma_start(kin, k8[:, ha:hb + 1, 0:SQ])
            ops.append((0, send))

            def cc():
                nc.gpsimd.collective_compute(
                    "AllGather", mybir.AluOpType.bypass,
                    replica_groups=GROUPS,
                    ins=[kin.opt()], outs=[kout.opt()])
            ops.append((0, cc))

            def load():
                nc.sync.dma_start(k8[:, ha:hb + 1, 0:SQ], kout[0])
                nc.sync.dma_start(k8[:, ha:hb + 1, SQ:S], kout[1])
            ops.append((0, load))
            return ops

        def v_exchange_ops(g):
            """Pairwise AllGather of vhi/vlo[:, g, 0:KCP//2] -> full kcp."""
            HB = KCP // 2
            vin = nc.dram_tensor(f"vin{rep}_{g}", [P, 2, HB, 2, 4, 68], F8).ap()
            vout = nc.dram_tensor(f"vout{rep}_{g}", [2, P, 2, HB, 2, 4, 68],
                                  F8).ap()
            ops = []

            def send():
                nc.sync.dma_start(vin[:, 0], vhi[:, g, 0:HB])
                nc.sync.dma_start(vin[:, 1], vlo[:, g, 0:HB])
            ops.append((0, send))

            def cc():
                nc.gpsimd.collective_compute(
                    "AllGather", mybir.AluOpType.bypass,
                    replica_groups=GROUPS,
                    ins=[vin.opt()], outs=[vout.opt()])
            ops.append((0, cc))

            def load():
                nc.sync.dma_start(vhi[:, g, 0:HB], vout[0, :, 0])
                nc.sync.dma_start(vhi[:, g, HB:KCP], vout[1, :, 0])
                nc.sync.dma_start(vlo[:, g, 0:HB], vout[0, :, 1])
                nc.sync.dma_start(vlo[:, g, HB:KCP], vout[1, :, 1])
            ops.append((0, load))
            return ops

        def proj_qk_ops(hp):
            """Q (query-half 0) + K own-half projection + K exchange."""
            ops = []
            st = {}

            def wload():
                wsb_t[hp] = w1.tile([P, 2, EC, P], BF, tag="wqk",
                                    name=f"wqk{hp}")
                nc.sync.dma_start(wsb_t[hp][:], wqk_d[hp])
            ops.append((0, wload))

            # Q projection, query-half 0 only (half 1 runs during qh 1)
            def qalloc():
                st["pq"] = ps_p.tile([P, 512], F32, tag="PROJ",
                                     name=f"pq{hp}_0")
            ops.append((0, qalloc))
            for ec in range(EC):
                def qmm(ec=ec):
                    nc.tensor.matmul(
                        st["pq"][:], wsb_t[hp][:, 0, ec],
                        xb[:, ec, 0:512],
                        start=(ec == 0), stop=(ec == EC - 1))
                ops.append((213, qmm))

            def qcopy():
                _q_split(hp, st["pq"], slice(0, 512))
            ops.append((0, qcopy))

            # K projection: own half only (two 512-key chunks)
            for nk in range(2):
                def kalloc(nk=nk):
                    st["pk"] = ps_p.tile([P, 512], F32, tag="PROJ",
                                         name=f"pk{hp}_{nk}")
                ops.append((0, kalloc))
                for ec in range(EC):
                    def kmm(ec=ec, nk=nk):
                        nc.tensor.matmul(
                            st["pk"][:], wsb_t[hp][:, 1, ec],
                            xb[:, ec, nk * 512:(nk + 1) * 512],
                            start=(ec == 0), stop=(ec == EC - 1))
                    ops.append((213, kmm))

                def kcopy(nk=nk):
                    ha, hb = 2 * hp, 2 * hp + 1
                    ck = slice(nk * 512, (nk + 1) * 512)
                    pk = st["pk"]
                    nc.vector.tensor_copy(k8[0:64, ha, ck], pk[0:64])
                    nc.vector.tensor_sub(
                        k8[64:128, ha, ck], pk[0:64], k8[0:64, ha, ck])
                    nc.vector.tensor_copy(k8[64:128, hb, ck], pk[64:128])
                    nc.vector.tensor_sub(
                        k8[0:64, hb, ck], pk[64:128], k8[64:128, hb, ck])
                ops.append((0, kcopy))
            ops += k_exchange_ops(hp)
            return ops

        def proj_v_ops(g):
            """V projection for group g (heads 4g..4g+3), own half + exchange."""
            ops = []
            st = {}
            for tc_i in range(TCH):
                def valloc(tc_i=tc_i):
                    st["pv"] = ps_p.tile([P, 512], F32, tag="PROJ",
                                         name=f"pv{g}_{tc_i}")
                ops.append((0, valloc))
                for ec in range(EC):
                    def vmm(ec=ec, tc_i=tc_i):
                        nc.tensor.matmul(
                            st["pv"][:, :256],
                            xb[:, ec, tc_i * P:(tc_i + 1) * P],
                            wv[:, ec, g * 256:(g + 1) * 256],
                            start=(ec == 0), stop=(ec == EC - 1))
                    ops.append((107, vmm))

                def vcopy(tc_i=tc_i):
                    dst = (slice(None), g, tc_i // 2, tc_i % 2,
                           slice(None), slice(0, 64))
                    src = st["pv"][:, :256].rearrange(
                        "p (h d) -> p h d", h=4)
                    nc.vector.tensor_copy(vhi[dst], src)
                    nc.vector.tensor_sub(vlo[dst], src, vhi[dst])
                ops.append((0, vcopy))
            ops += v_exchange_ops(g)
            return ops

        def proj_q1_ops(hp):
            """Q projection for query-half 1 (weight chunk re-fetched: the
            streamed wqk tile for hp has been recycled by then)."""
            ops = []
            st = {}

            def wload():
                st["wq1"] = w1.tile([P, EC, P], BF, tag="wq1",
                                    name=f"wq1_{hp}")
                nc.sync.dma_start(st["wq1"][:], wqk_d[hp, :, 0])
            ops.append((0, wload))

            def qalloc():
                st["pq"] = ps_p.tile([P, 512], F32, tag="PROJ",
                                     name=f"pq{hp}_1")
            ops.append((0, qalloc))
            for ec in range(EC):
                def qmm(ec=ec):
                    nc.tensor.matmul(
                        st["pq"][:], st["wq1"][:, ec], xb[:, ec, 512:1024],
                        start=(ec == 0), stop=(ec == EC - 1))
                ops.append((213, qmm))

            def qcopy():
                _q_split(hp, st["pq"], slice(512, 1024))
            ops.append((0, qcopy))
            return ops

        def outproj_ops(qc, nfs=(0, 1)):
            ops = []
            st = {}
            for nf in nfs:
                def oalloc(nf=nf):
                    st["py"] = ps_p.tile([P, 512], F32, tag="PROJ",
                                         name=f"py{qc}_{nf}")
                ops.append((0, oalloc))
                for o in range(EC):
                    def omm(o=o, nf=nf):
                        nc.tensor.matmul(
                            st["py"][:], onT[:, o, qc * P:(qc + 1) * P],
                            wo[:, o, nf * 512:(nf + 1) * 512],
                            start=(o == 0), stop=(o == EC - 1))
                    ops.append((213, omm))

                def ostore(nf=nf):
                    y_sb = w.tile([P, 512], BF, tag="ysb")
                    nc.vector.tensor_add(
                        y_sb[:], st["py"][:],
                        bo_rep[:, nf * 512:(nf + 1) * 512])
                    nc.sync.dma_start(
                        y_d[qc * P:(qc + 1) * P, nf * 512:(nf + 1) * 512],
                        y_sb[:])
                ops.append((0, ostore))
            return ops

        ysum_t = {}
        O_SPLIT = 5

        def outproj_part1_ops(qc, nf):
            """First O_SPLIT contraction chunks of outproj(qc, nf), banked
            into an SBUF partial (seeded with the bias)."""
            ops = []
            st = {}

            def oalloc():
                st["py"] = ps_p.tile([P, 512], F32, tag="PROJ",
                                     name=f"pyA{qc}_{nf}")
            ops.append((0, oalloc))
            for o in range(O_SPLIT):
                def omm(o=o):
                    nc.tensor.matmul(
                        st["py"][:], onT[:, o, qc * P:(qc + 1) * P],
                        wo[:, o, nf * 512:(nf + 1) * 512],
                        start=(o == 0), stop=(o == O_SPLIT - 1))
                ops.append((213, omm))

            def obank():
                ysum_t[(qc, nf)] = ys.tile([P, 512], BF, tag="ysum",
                                           name=f"ys{qc}_{nf}")
                nc.vector.tensor_add(
                    ysum_t[(qc, nf)][:], st["py"][:],
                    bo_rep[:, nf * 512:(nf + 1) * 512])
            ops.append((0, obank))
            return ops

        def outproj_part2_ops(qc, nf):
            ops = []
            st = {}

            def oalloc():
                st["py"] = ps_p.tile([P, 512], F32, tag="PROJ",
                                     name=f"pyB{qc}_{nf}")
            ops.append((0, oalloc))
            for o in range(O_SPLIT, EC):
                def omm(o=o):
                    nc.tensor.matmul(
                        st["py"][:], onT[:, o, qc * P:(qc + 1) * P],
                        wo[:, o, nf * 512:(nf + 1) * 512],
                        start=(o == O_SPLIT), stop=(o == EC - 1))
                ops.append((213, omm))

            def ostore():
                y_sb = w.tile([P, 512], BF, tag="ysb")
                nc.vector.tensor_add(
                    y_sb[:], st["py"][:], ysum_t[(qc, nf)][:])
                nc.sync.dma_start(
                    y_d[qc * P:(qc + 1) * P, nf * 512:(nf + 1) * 512],
                    y_sb[:])
            ops.append((0, ostore))
            return ops

        def emit_attn(qh, hp, aux):
            """Attention for (query half qh, head pair hp); aux ops are
            interleaved at kc granularity to fill PE gaps."""
            g = hp // 2
            ha, hb = 2 * (hp % 2), 2 * (hp % 2) + 1
            qs = slice(qh * 512, (qh + 1) * 512)
            po_a = ps_o.tile([68, 512], F32, tag="po")
            po_b = ps_o.tile([68, 512], F32, tag="po")
            total_cost = sum(c for c, _ in aux) or 1
            n_emit = 0
            cum = 0
            uts = {}

            def emit_pv(kcp):
                ut = uts.pop(kcp)
                for po, h in ((po_a, ha), (po_b, hb)):
                    nc.tensor.matmul(
                        po[:], vhi[:, g, kcp, :, h], ut[:, :, h % 2],
                        start=(kcp == 0), stop=False, perf_mode=DR)
                    nc.tensor.matmul(
                        po[:], vlo[:, g, kcp, :, h], ut[:, :, h % 2],
                        start=False, stop=(kcp == KCP - 1), perf_mode=DR)

            for kc in range(TC):
                kcp, sub = kc // 2, kc % 2
                if sub == 0:
                    uts[kcp] = ut_pool.tile([P, 2, 2, 512], F8, tag="ut", name="ut")
                ut = uts[kcp]
                # PV for pair j is emitted one pair late so it never blocks
                # the scores that feed exp; the sc pool's WAR lag (2 kc)
                # then paces the PE score stream to ACT's rate.
                sc = ps_s.tile([P, 2, 512], F32, tag="S", name="sc")
                ks = slice(kc * P, (kc + 1) * P)
                nc.tensor.matmul(
                    sc[:, 0],
                    k8[:, 2 * hp, ks][:, None, :].to_broadcast([P, 2, P]),
                    q8[:, :, 2 * hp, qs],
                    start=True, stop=True, perf_mode=DR)
                nc.tensor.matmul(
                    sc[:, 1],
                    k8[:, 2 * hp + 1, ks][:, None, :].to_broadcast([P, 2, P]),
                    q8[:, :, 2 * hp + 1, qs],
                    start=True, stop=True, perf_mode=DR)
                nc.scalar.activation(ut[:, sub], sc[:], AF.Exp,
                                     scale=EXP_SCALE)
                if kc % 2 == 1 and kcp >= 1:
                    emit_pv(kcp - 1)
                target = total_cost * min(kc + 2, TC) // TC
                while n_emit < len(aux) and cum < target:
                    cum += aux[n_emit][0]
                    aux[n_emit][1]()
                    n_emit += 1
            emit_pv(KCP - 1)
            while n_emit < len(aux):
                aux[n_emit][1]()
                n_emit += 1

            # normalize; row 64 of po_x is the softmax denominator.
            # partition_broadcast only writes correctly from base 0:
            # broadcast into a full tile, slice at read time.
            rcp = rp.tile([1, 1024], BF, tag="rcp")
            nc.vector.reciprocal(rcp[:, 0:512], po_a[64:65, :])
            nc.vector.reciprocal(rcp[:, 512:1024], po_b[64:65, :])
            brec = w.tile([P, 1024], BF, tag="brec")
            nc.gpsimd.partition_broadcast(brec[:], rcp[:])
            nc.vector.tensor_mul(
                onT[0:64, hp, qs], po_a[0:64, :], brec[0:64, 0:512])
            nc.vector.tensor_mul(
                onT[64:128, hp, qs], po_b[0:64, :], brec[64:128, 512:1024])

        # ---- schedule ----
        # Prologue: wqk0 DMA first, xb streamed per e-chunk, then hp0's
        # Q(half 0) / K-own / K-exchange and g0's V-own / V-exchange, so the
        # first attention block's collectives land while its scores stream.
        qk0 = proj_qk_ops(0)
        qk0[0][1]()                           # wqk[0] DMA
        for ec in range(EC):                  # xb split so PE starts early
            nc.sync.dma_start(xb[:, ec], xb_d[:, ec])
        nc.sync.dma_start(wv[:], wv_d)
        bo_one = const.tile([1, E], BF)
        nc.sync.dma_start(bo_one[:], bo_d)
        nc.gpsimd.partition_broadcast(bo_rep[:], bo_one[:])
        for _, op in qk0[1:]:                 # Q half 0 + K own + exchange
            op()
        for _, op in proj_v_ops(0):           # V own g0 + exchange
            op()

        # query-half 0 blocks: the remaining Q/K-own projections (+their
        # exchanges), V groups 1-3 (+exchanges) and the deferred Q-half-1
        # projections ride inside the blocks, deadline-ordered: everything a
        # block consumes is emitted at least one block earlier.
        for hp in range(HP):
            if hp == 0:
                aux = proj_qk_ops(1) + proj_v_ops(1)
            elif hp == 1:
                aux = proj_qk_ops(2) + proj_qk_ops(3)
            elif hp == 2:
                aux = proj_qk_ops(4) + proj_v_ops(2)
            elif hp == 3:
                aux = proj_qk_ops(5) + proj_qk_ops(6)
            elif hp == 4:
                aux = proj_qk_ops(7) + proj_v_ops(3)
            elif hp == 5:
                aux = proj_q1_ops(0) + proj_q1_ops(1)
            elif hp == 6:
                aux = proj_q1_ops(2) + proj_q1_ops(3)
            else:
                aux = proj_q1_ops(4) + proj_q1_ops(5)
            emit_attn(0, hp, aux)

        # query-half 1 blocks: absorb the output projection of query-half 0
        # (one 512-col chunk per block), Q half-1 of hp 6/7, the wo load, and
        # the first part of query-half 1's own output projection.
        def wo_load():
            nc.sync.dma_start(wo[:], wo_d)
        for hp in range(HP):
            aux = [(0, wo_load)] if hp == 0 else []
            aux += outproj_ops(hp // 2, nfs=(hp % 2,))
            if hp == 0:
                aux += proj_q1_ops(6)
            elif hp == 1:
                aux += proj_q1_ops(7)
            if hp in (5, 6):
                qc = 4 + 2 * (hp - 5)
                for nf in range(2):
                    aux += outproj_part1_ops(qc, nf)
                    aux += outproj_part1_ops(qc + 1, nf)
            emit_attn(1, hp, aux)
        for qc in range(4, QC):
            for nf in range(2):
                for _, op in outproj_part2_ops(qc, nf):
                    op()


def _build_kernel(reps=1):
    nc = bacc.Bacc("TRN2", target_bir_lowering=False, debug=False,
                   num_devices=N_CORES)
    xb_d = nc.dram_tensor("xb", [P, EC, SQ], BF, kind="ExternalInput").ap()
    wqk_d = nc.dram_tensor("wqk", [HP, P, 2, EC, P], BF,
                           kind="ExternalInput").ap()
    wv_d = nc.dram_tensor("wv", [P, EC, E], BF, kind="ExternalInput").ap()
    wo_d = nc.dram_tensor("wo", [P, EC, E], BF, kind="ExternalInput").ap()
    bo_d = nc.dram_tensor("bo", [1, E], BF, kind="ExternalInput").ap()
    y_d = nc.dram_tensor("y", [SQ, E], BF, kind="ExternalOutput").ap()

    with tile.TileContext(nc) as tc:
        for _ in range(reps):
            _emit(nc, tc, xb_d, wqk_d, wv_d, wo_d, bo_d, y_d)
    nc.compile()
    return nc


_NC_CACHE = None


def make_in_maps(x, Wq, Wk, Wv, Wo, bo):
    x = np.asarray(x, np.float32)
    bf16 = ml_dtypes.bfloat16

    def pack_w(Wt, scale):
        # [E_in, E_out] -> [P, EC, E_out]
        return np.ascontiguousarray(
            (Wt * scale).reshape(EC, P, E).transpose(1, 0, 2)).astype(bf16)

    wqt = pack_w(np.asarray(Wq, np.float32).T, QK_SCALE)   # [P, EC, E]
    wkt = pack_w(np.asarray(Wk, np.float32).T, QK_SCALE)
    wv_ = pack_w(np.asarray(Wv, np.float32).T, WV_SCALE)
    wo_ = pack_w(np.asarray(Wo, np.float32).T, 1.0 / WV_SCALE)

    # wqk: [HP, P, 2, EC, P]
    wqk = np.empty((HP, P, 2, EC, P), bf16)
    for hp in range(HP):
        wqk[hp, :, 0] = wqt[:, :, hp * P:(hp + 1) * P]
        wqk[hp, :, 1] = wkt[:, :, hp * P:(hp + 1) * P]
    wqk = np.ascontiguousarray(wqk)
    bo_ = np.ascontiguousarray(np.asarray(bo, np.float32).reshape(1, E)).astype(bf16)

    in_maps = []
    for c in range(N_CORES):
        b, half = c // 2, c % 2
        x_own = x[b, half * SQ:(half + 1) * SQ]     # [SQ, E]
        xb8 = np.ascontiguousarray(
            x_own.T.reshape(EC, P, SQ).transpose(1, 0, 2)).astype(bf16)
        in_maps.append({"xb": xb8, "wqk": wqk, "wv": wv_, "wo": wo_,
                        "bo": bo_})
    return in_maps


def get_nc(reps=1):
    global _NC_CACHE
    if _NC_CACHE is None:
        _NC_CACHE = {}
    if reps not in _NC_CACHE:
        _NC_CACHE[reps] = _build_kernel(reps)
    return _NC_CACHE[reps]


def kernel(x, Wq, Wk, Wv, Wo, bo):
    nc = get_nc()
    in_maps = make_in_maps(x, Wq, Wk, Wv, Wo, bo)
    res = run_bass_kernel_spmd(nc, in_maps, core_ids=list(range(N_CORES)))
    out = np.empty((B, S, E), np.float32)
    for c in range(N_CORES):
        b, half = c // 2, c % 2
        out[b, half * SQ:(half + 1) * SQ, :] = \
            res.results[c]["y"].astype(np.float32)
    return out
